# revision 13
# baseline (speedup 1.0000x reference)
"""Cross-attention Trainium2 kernel (8 NeuronCores, SPMD), v6.

Reference computation (per full batch):
  q = x @ Wq + bq;  k = enc @ Wk + bk;  v = enc @ Wv + bv
  att = softmax((q k^T) / sqrt(D));  y = (att v) @ Wo + bo

Sharding: B(=4) x head-group(=2) -> 8 cores. Each core handles one batch
element, 8 of the 16 heads, and ALL 2048 query tokens; it returns the
f32 partial out-projection for its 512 channels. Host sums the two
head-group partials per batch + bo. vs v5's B x T-half split this
halves the duplicated K/V projections and removes the K=128
zero-padding of scores entirely.

Key mechanisms (validated by microbench on this HW):
  * Scores run as row-tiled K=64 matmul PAIRS: even head's k/q on
    partitions 0-63 (PE tile (0,0)), odd head's on 64-127 (tile
    (64,0)), outputs to different PSUM banks -> the two matmuls execute
    CONCURRENTLY (measured 155ns/MM vs 332 serial). The natural channel
    layout already interleaves head pairs this way - no data movement.
  * PE tile-config switches (64-row vs 128-row) cost ~65ns when
    alternating, ~0 when batched: each (pair,tn) unit runs all 8 score
    pairs as one 64-cfg block, then attv + projection thunks as one
    128-cfg block.
  * All matmul operands bf16; N=512 moving (PSUM f32 bank cap,
    walrus rejects 1024). Projections as 8-deep accumulation chains
    (242-261ns/MM measured), out-proj 4-deep.
  * v is built in the att@v lhsT layout vS[u] [128, 2, 8*65] (65-col
    groups per head: [v_h | ones]); psum row 64 of ya = softmax
    denominator for free.
  * Softmax: exp on ACT (psum [128,1024] -> bf16, fused 1/sqrt(D)
    scale), one instruction per (pair,sc) covering both heads;
    reciprocal_approx_fast + gpsimd partition_broadcast + DVE mult.
  * Schedule: fine-grained software pipeline over 16 units
    (tn(4) x head-pair(4)). Each unit weaves 4 "visits" of 2 score
    pairs (64-cfg) with chunks of a 128-cfg micro-op queue holding the
    PREVIOUS unit's att@v chains + projection groups (K/V/Q/O spread
    across units per a hand-placed table in unit_queues()). This keeps
    PE fed while ACT paces the 2-deep scores PSUM, and keeps ACT fed
    across unit boundaries -- v6's block schedule ping-ponged PE<->ACT
    at ~5us/unit. Prologue = 20 HAM warm-up matmuls + K(cp0)+V(0,1)+
    Q(cp0,tn0) under the DMA lead-in; epilogue = attv(last) + out-proj
    of tn3.
  * PSUM: scores 2x[128,1024] (4 banks) + ya [65,1024] (2) +
    projection/out psum 2x[128,512] (2) = exactly 8 banks.

Engine budget per core: PE ~205us busy (the bottleneck: 384 proj slots
+ 128 score pair-slots + 256 attv slots at ~240-310ns measured), ACT
~135us (128 exps of [128,1024] at ~1.05us), DVE ~60us, out-DMA 8MB f32
(host reduces). Measured paired-median ~255us (v5 baseline: 313.7us).
"""

import sys

sys.path.insert(0, "/opt/trn_rl_repo")

import numpy as np

import concourse.bass as bass  # noqa: E402,F401
import concourse.tile as tile  # noqa: E402
from concourse import bacc, mybir  # noqa: E402

F32 = mybir.dt.float32
BF16 = mybir.dt.bfloat16
AF = mybir.ActivationFunctionType

P = 128          # partitions
TC = 2048        # query tokens per core (all of T)
T2 = 1024        # kv sequence length
C = 1024         # embed dim
HC = 8           # heads per core
D = 64           # head dim
NCH = C // P     # 8 input-channel chunks
CP = 4           # output channel-pair chunks (512 cols / 128)
NS = T2 // P     # 8 kv-position chunks
TN = 512         # matmul moving-dim tile
NTN = TC // TN   # 4 query-token chunks
G = D + 1        # v-group stride in vS (64 v cols + ones col)
SCALE = 1.0 / np.sqrt(D)

N_CORES = 8
B_FULL, T_FULL = 4, 2048


def build_program(loop_iters=None, debug=False):
    """loop_iters: if set, wrap the body in a For_i hardware loop (timing)."""
    nc = bacc.Bacc("TRN2", target_bir_lowering=False, debug=False,
                   num_devices=N_CORES)

    aps = {}
    aps["xT"] = nc.dram_tensor("xT", [P, NTN, NCH, TN], BF16,
                               kind="ExternalInput").ap()
    aps["encT"] = nc.dram_tensor("encT", [P, NCH, T2], BF16,
                                 kind="ExternalInput").ap()
    for name in ("wq", "wk", "wv"):
        aps[name] = nc.dram_tensor(name, [P, NCH, CP * P], BF16,
                                   kind="ExternalInput").ap()
    aps["wo"] = nc.dram_tensor("wo", [P, CP, C], BF16,
                               kind="ExternalInput").ap()
    for name in ("bqc", "bkc"):
        aps[name] = nc.dram_tensor(name, [P, CP], F32,
                                   kind="ExternalInput").ap()
    aps["bv"] = nc.dram_tensor("bv", [CP * P], F32,
                               kind="ExternalInput").ap()
    out = nc.dram_tensor("out", [TC, C], F32, kind="ExternalOutput").ap()

    dbg = None
    if debug:
        dbg = {}
        for name, shape, dt in (
                ("d_kS", [P, CP, T2], BF16), ("d_qT", [P, CP, TC], BF16),
                ("d_yT", [P, CP, TC], BF16),
                ("d_vS0", [P, 2, HC * G], BF16),
                ("d_px0", [P, 2 * TN], BF16)):
            dbg[name] = nc.dram_tensor(name, shape, dt,
                                       kind="ExternalOutput").ap()

    with tile.TileContext(nc) as tc:
        if loop_iters is not None:
            with tc.For_i(0, loop_iters, 1):
                _emit(nc, tc, aps, out)
        else:
            _emit(nc, tc, aps, out, dbg)

    nc.compile()
    return nc


def _row(ap):
    return ap.rearrange("(a c) -> a c", a=1)


def _emit(nc, tc, aps, out, dbg=None):
    from contextlib import ExitStack

    with ExitStack() as S:
        pIn = S.enter_context(tc.tile_pool(name="pIn", bufs=1))

        # ---- persistent tiles
        wo = pIn.tile([P, CP, C], BF16, tag="wo", name="wo")
        kS = pIn.tile([P, CP, T2], BF16, tag="kS", name="kS")
        qT = pIn.tile([P, CP, TC], BF16, tag="qT", name="qT")
        yT = pIn.tile([P, CP, TC], BF16, tag="yT", name="yT")
        vS = [pIn.tile([P, 2, HC * G], BF16, tag=f"vS_{u}", name=f"vS_{u}")
              for u in range(NS // 2)]
        xTs = pIn.tile([P, NTN, NCH, TN], BF16, tag="xTs", name="xTs")
        wq = pIn.tile([P, NCH, CP * P], BF16, tag="wq", name="wq")
        bqc = pIn.tile([P, CP], F32, tag="bqc", name="bqc")

        psP = S.enter_context(tc.tile_pool(name="psP", bufs=2, space="PSUM"))
        psS = S.enter_context(tc.tile_pool(name="psS", bufs=2, space="PSUM"))
        psY = S.enter_context(tc.tile_pool(name="psY", bufs=1, space="PSUM"))

        state = {}

        def open_proj_scope(S2):
            # DMA need-order: K-proj inputs first (encT+wk), then wv, then
            # xT tn0 + wq (Q-proj), then rest of xT, wo last.
            pTmp = S2.enter_context(tc.tile_pool(name="pTmp", bufs=1))
            bkc = pTmp.tile([P, CP], F32, tag="bkc", name="bkc")
            nc.sync.dma_start(out=bkc, in_=aps["bkc"])
            nc.scalar.dma_start(out=bqc, in_=aps["bqc"])
            bv_row = pTmp.tile([1, CP * P], F32, tag="bv_row", name="bv_row")
            nc.sync.dma_start(out=bv_row, in_=_row(aps["bv"]))

            encT = pTmp.tile([P, NCH, T2], BF16, tag="encT", name="encT")
            wk = pTmp.tile([P, NCH, CP * P], BF16, tag="wk", name="wk")
            wv = pTmp.tile([P, NCH, CP * P], BF16, tag="wv", name="wv")
            for kc in range(NCH):
                # encT split across both queues, wk woven in kc order so
                # the K-proj chain starts as soon as chunk 0 lands
                qe = nc.sync if kc % 2 == 0 else nc.scalar
                qw = nc.scalar if kc % 2 == 0 else nc.sync
                qe.dma_start(out=encT[:, kc, :], in_=aps["encT"][:, kc, :])
                qw.dma_start(out=wk[:, kc, :], in_=aps["wk"][:, kc, :])
            for kc in range(NCH):
                q = nc.sync if kc >= 4 else nc.scalar
                q.dma_start(out=wv[:, kc, :], in_=aps["wv"][:, kc, :])
            for kc in range(NCH):
                nc.sync.dma_start(out=xTs[:, 0, kc, :],
                                  in_=aps["xT"][:, 0, kc, :])
                nc.scalar.dma_start(out=wq[:, kc, :], in_=aps["wq"][:, kc, :])
            for tn in range(1, NTN):
                for kc in range(NCH):
                    q = nc.sync if (kc + tn) % 2 else nc.scalar
                    q.dma_start(out=xTs[:, tn, kc, :],
                                in_=aps["xT"][:, tn, kc, :])
            nc.scalar.dma_start(out=wo, in_=aps["wo"])

            bvb = pTmp.tile([P, CP * P], F32, tag="bvb", name="bvb")
            nc.gpsimd.partition_broadcast(bvb, bv_row)
            state.update(encT=encT, wk=wk, wv=wv, bkc=bkc, bvb=bvb)

        def chain_ops(n_mm, mk_ps, mm, tail):
            """Micro-op list for an accumulation-chain group: n_mm matmul
            callables sharing one psum tile (allocated at first call) plus a
            trailing non-PE consumer."""
            box = {}

            def mm_i(i):
                if i == 0:
                    box["ps"] = mk_ps()
                mm(box["ps"], i)
            return ([lambda i=i: mm_i(i) for i in range(n_mm)]
                    + [lambda: tail(box["ps"])])

        def k_ops(cp, stn):
            def mm(ps, kc):
                nc.tensor.matmul(
                    ps, state["wk"][:, kc, cp * P:(cp + 1) * P],
                    state["encT"][:, kc, stn * TN:(stn + 1) * TN],
                    start=(kc == 0), stop=(kc == NCH - 1))

            def tail(ps):
                nc.vector.tensor_scalar_add(
                    kS[:, cp, stn * TN:(stn + 1) * TN], ps,
                    state["bkc"][:, cp:cp + 1])
            return chain_ops(
                NCH, lambda: psP.tile([P, TN], F32, tag="pp", name="psK"),
                mm, tail)

        def q_ops(cp, tn):
            def mm(ps, kc):
                nc.tensor.matmul(
                    ps, wq[:, kc, cp * P:(cp + 1) * P], xTs[:, tn, kc, :],
                    start=(kc == 0), stop=(kc == NCH - 1))

            def tail(ps):
                nc.vector.tensor_scalar_add(
                    qT[:, cp, tn * TN:(tn + 1) * TN], ps, bqc[:, cp:cp + 1])
            return chain_ops(
                NCH, lambda: psP.tile([P, TN], F32, tag="pp", name="psQ"),
                mm, tail)

        def v_ops(sc):
            u, j = sc // 2, sc % 2

            def mm(ps, kc):
                nc.tensor.matmul(
                    ps, state["encT"][:, kc, sc * P:(sc + 1) * P],
                    state["wv"][:, kc, :],
                    start=(kc == 0), stop=(kc == NCH - 1))

            def tail(ps):
                dst = vS[u][:, j, :].rearrange(
                    "p (h g) -> p h g", g=G)[:, :, 0:D]
                srcp = ps.rearrange("p (h g) -> p h g", g=D)
                bsrc = state["bvb"].rearrange("p (h g) -> p h g", g=D)
                nc.vector.tensor_add(dst, srcp, bsrc)
            return chain_ops(
                NCH, lambda: psP.tile([P, TN], F32, tag="pp", name="psV"),
                mm, tail)

        def o_ops(tp, nn):
            # f32 partial out-projection (host adds the other head-group
            # partial + bo); nn==1 tail also DMAs the finished row block.
            def mm(ps, kc):
                nc.tensor.matmul(
                    ps, yT[:, kc, tp * P:(tp + 1) * P],
                    wo[:, kc, nn * TN:(nn + 1) * TN],
                    start=(kc == 0), stop=(kc == CP - 1))

            def tail(ps):
                if nn == 0:
                    state[f"osb{tp}"] = state["pO"].tile(
                        [P, C], F32, tag="o", bufs=2, name=f"o{tp}")
                o_sb = state[f"osb{tp}"]
                nc.vector.tensor_copy(o_sb[:, nn * TN:(nn + 1) * TN], ps)
                if nn == 1:
                    nc.scalar.dma_start(
                        out=out[tp * P:(tp + 1) * P, :], in_=o_sb)
            return chain_ops(
                CP, lambda: psP.tile([P, TN], F32, tag="pp", name="psO"),
                mm, tail)

        def attv_ops(u):
            """att@v chains + normalization for unit u (consumes px[u])."""
            tn, cp = u // CP, u % CP
            ts = slice(tn * TN, (tn + 1) * TN)
            h0, h1 = 2 * cp, 2 * cp + 1
            box = {}
            ops = []

            def av(sc, half, h):
                if sc == 0 and half == 0:
                    box["ya"] = psY.tile([G, 2 * TN], F32, tag="ya",
                                         name="ya")
                uu, j = sc // 2, sc % 2
                nc.tensor.matmul(
                    box["ya"][:, half * TN:(half + 1) * TN],
                    vS[uu][:, j, h * G:(h + 1) * G],
                    state["px"][u][sc][:, half * TN:(half + 1) * TN],
                    start=(sc == 0), stop=(sc == NS - 1))
            for sc in range(NS):
                ops.append(lambda sc=sc: av(sc, 0, h0))
                ops.append(lambda sc=sc: av(sc, 1, h1))

            def norm():
                ya = box["ya"]
                den = pRc.tile([1, 2 * TN], F32, tag="den", bufs=2,
                               name="den")
                nc.vector.tensor_copy(den, ya[D:D + 1, :])
                rc = pRc.tile([1, 2 * TN], F32, tag="rc", bufs=2, name="rc")
                nc.vector.reciprocal_approx_fast(rc, den)
                bc = pBc.tile([D, 2 * TN], F32, tag="bc", bufs=2, name="bc")
                nc.gpsimd.partition_broadcast(bc, rc)
                nc.vector.tensor_mul(yT[0:D, cp, ts], ya[0:D, 0:TN],
                                     bc[:, 0:TN])
                nc.vector.tensor_mul(yT[D:P, cp, ts], ya[0:D, TN:2 * TN],
                                     bc[:, TN:2 * TN])
                del state["px"][u]
            ops.append(norm)
            return ops

        def emit_unit(u, queue):
            """Scores for unit u woven with 128-cfg micro-ops from queue.

            Visits issue 2 score-pairs (4 MMs, 64-cfg) + their exps, then
            drain a slice of the queue (attv of unit u-1 + projection
            groups), keeping PE fed while ACT paces the scores banks."""
            tn, cp = u // CP, u % CP
            ts = slice(tn * TN, (tn + 1) * TN)
            px_list = []
            state["px"][u] = px_list
            import os
            n_visits = NS // int(os.environ.get("VIS", "2"))
            per = NS // n_visits
            for v in range(n_visits):
                for sc in range(per * v, per * v + per):
                    ps = psS.tile([P, 2 * TN], F32, tag="ps", bufs=2,
                                  name="psS")
                    nc.tensor.matmul(
                        ps[:, 0:TN], kS[0:D, cp, sc * P:(sc + 1) * P],
                        qT[0:D, cp, ts], start=True, stop=True)
                    nc.tensor.matmul(
                        ps[:, TN:2 * TN], kS[D:P, cp, sc * P:(sc + 1) * P],
                        qT[D:P, cp, ts], start=True, stop=True)
                    px = pPx.tile([P, 2 * TN], BF16, tag="px", bufs=18,
                                  name="px")
                    nc.scalar.activation(px, ps, AF.Exp, scale=float(SCALE))
                    px_list.append(px)
                n_chunk = (len(queue) + n_visits - v - 1) // (n_visits - v)
                for _ in range(n_chunk):
                    queue.pop(0)()
            while queue:
                queue.pop(0)()

        # ================= schedule =================
        with ExitStack() as S2:
            open_proj_scope(S2)
            pPx = S2.enter_context(tc.tile_pool(name="pPx", bufs=18))
            pRc = S2.enter_context(tc.tile_pool(name="pRc", bufs=2))
            pBc = S2.enter_context(tc.tile_pool(name="pBc", bufs=2))
            state["pO"] = S2.enter_context(tc.tile_pool(name="pO", bufs=2))
            state["px"] = {}

            for u in range(NS // 2):
                ones_dst = vS[u].rearrange(
                    "p two (h g) -> p (two h) g", g=G)[:, :, D:D + 1]
                nc.vector.memset(ones_dst, 1.0)

            # HAM warm-up: dummy matmuls on uninitialized SBUF while the
            # first DMAs land, so the PE clock gate is at 2.4GHz (not the
            # cold 1.2GHz) when real work starts. Results are discarded.
            for i in range(20):
                ps = psS.tile([P, 2 * TN], F32, tag="ps", bufs=2,
                              name="psWarm")
                nc.tensor.matmul(ps[:, 0:TN], qT[:, 0, 0:P],
                                 qT[:, 0, 0:TN], start=True, stop=True)

            # prologue: the minimal serial work before unit 0's scores,
            # ordered by DMA arrival (encT+wk first, wv, xT tn0 last)
            for op in (k_ops(0, 0) + k_ops(0, 1) + v_ops(0) + v_ops(1)
                       + q_ops(0, 0)):
                op()

            # per-unit woven thunk groups (pre = before attv(u-1) in the
            # queue -- needed for V groups feeding that attv; post = after).
            Qs, Ks, Vs, Os = q_ops, k_ops, v_ops, o_ops

            def unit_queues():
                q = {}
                q[0] = (Vs(2) + Vs(3) + Ks(1, 0) + Ks(1, 1) + Qs(1, 0), [])
                q[1] = (Vs(4) + Vs(5) + Vs(6) + Vs(7),
                        Ks(2, 0) + Ks(2, 1) + Qs(2, 0))
                q[2] = ([], Ks(3, 0) + Ks(3, 1) + Qs(3, 0))
                q[3] = ([], Qs(0, 1))
                q[4] = ([], Qs(1, 1) + Qs(2, 1))
                q[5] = ([], Qs(3, 1) + Os(0, 0) + Os(0, 1))
                q[6] = ([], Os(1, 0) + Os(1, 1) + Qs(0, 2))
                q[7] = ([], Os(2, 0) + Os(2, 1))
                q[8] = ([], Os(3, 0) + Os(3, 1) + Qs(1, 2))
                q[9] = ([], Qs(2, 2) + Qs(3, 2) + Os(4, 0))
                q[10] = ([], Os(4, 1) + Os(5, 0) + Os(5, 1))
                q[11] = ([], Os(6, 0) + Os(6, 1) + Qs(0, 3))
                q[12] = ([], Os(7, 0) + Os(7, 1) + Qs(1, 3))
                q[13] = ([], Qs(2, 3) + Qs(3, 3) + Os(8, 0))
                q[14] = ([], Os(8, 1) + Os(9, 0) + Os(9, 1))
                q[15] = ([], Os(10, 0) + Os(10, 1) + Os(11, 0) + Os(11, 1))
                return q

            queues = unit_queues()
            for u in range(4 * CP):
                pre, post = queues[u]
                av = attv_ops(u - 1) if u >= 1 else []
                emit_unit(u, pre + av + post)

            if dbg is not None:
                nc.sync.dma_start(out=dbg["d_px0"], in_=state["px"][15][0])
                nc.sync.dma_start(out=dbg["d_vS0"], in_=vS[0])
                nc.sync.dma_start(out=dbg["d_kS"], in_=kS)
                nc.sync.dma_start(out=dbg["d_qT"], in_=qT)

            # epilogue: last unit's attv + the final token chunk's out-proj
            for op in attv_ops(4 * CP - 1):
                op()
            for tp in range(12, 16):
                for op in Os(tp, 0) + Os(tp, 1):
                    op()

            if dbg is not None:
                nc.sync.dma_start(out=dbg["d_yT"], in_=yT)


def make_in_maps(inputs):
    """Full fp32 inputs -> per-core input maps (host-side shard + layout)."""
    import ml_dtypes

    bf16 = ml_dtypes.bfloat16
    x = np.asarray(inputs["x"], dtype=np.float32)
    enc = np.asarray(inputs["enc_x"], dtype=np.float32)

    def chunked_T(a2d):
        # [rows, C] fp32 -> [128, NCH, rows] bf16: [p, c, r] = a[r, c*128+p]
        t = np.ascontiguousarray(
            a2d.T.reshape(NCH, P, a2d.shape[0]).transpose(1, 0, 2))
        return t.astype(bf16)

    encT = [chunked_T(enc[b]) for b in range(x.shape[0])]
    # xT[p, tn, kc, j] = x[b, tn*512+j, kc*128+p]
    xTb = [np.ascontiguousarray(
        x[b].reshape(NTN, TN, NCH, P).transpose(3, 0, 2, 1)).astype(bf16)
        for b in range(x.shape[0])]

    Wq = np.asarray(inputs["Wq"], np.float32)
    Wk = np.asarray(inputs["Wk"], np.float32)
    Wv = np.asarray(inputs["Wv"], np.float32)
    Wo = np.asarray(inputs["Wo"], np.float32)
    bq = np.asarray(inputs["bq"], np.float32)
    bk = np.asarray(inputs["bk"], np.float32)
    bv = np.asarray(inputs["bv"], np.float32)

    hgw = []
    for hg in range(2):
        cs = slice(hg * CP * P, (hg + 1) * CP * P)
        m = {}
        for name, W in (("wq", Wq), ("wk", Wk), ("wv", Wv)):
            m[name] = np.ascontiguousarray(
                W[:, cs].reshape(NCH, P, CP * P).transpose(1, 0, 2)
            ).astype(bf16)
        m["wo"] = np.ascontiguousarray(
            Wo[cs, :].reshape(CP, P, C).transpose(1, 0, 2)).astype(bf16)
        m["bqc"] = np.ascontiguousarray(bq[cs].reshape(CP, P).T)
        m["bkc"] = np.ascontiguousarray(bk[cs].reshape(CP, P).T)
        m["bv"] = np.ascontiguousarray(bv[cs])
        hgw.append(m)

    maps = []
    for core in range(N_CORES):
        b, hg = core // 2, core % 2
        m = {"xT": xTb[b], "encT": encT[b]}
        m.update(hgw[hg])
        maps.append(m)
    return maps


_CACHED = None


def _get_program():
    global _CACHED
    if _CACHED is None:
        _CACHED = build_program()
    return _CACHED


def kernel(**inputs):
    x = np.asarray(inputs["x"], dtype=np.float32)
    B, T, Cx = x.shape
    assert (B, T, Cx) == (B_FULL, T_FULL, C), (B, T, Cx)

    nc = _get_program()
    in_maps = make_in_maps(inputs)

    from concourse.bass_utils import run_bass_kernel_spmd
    res = None
    last_err = None
    for _attempt in range(3):
        try:
            res = run_bass_kernel_spmd(nc, in_maps,
                                       core_ids=list(range(N_CORES)))
            break
        except Exception as e:  # transient NRT/axon failures: retry
            last_err = e
    if res is None:
        raise last_err

    bo = np.asarray(inputs["bo"], np.float32)
    outp = np.empty((B, T, C), dtype=np.float32)
    for b in range(B):
        outp[b] = (res.results[2 * b]["out"] + res.results[2 * b + 1]["out"]
                   + bo)
    return outp


if __name__ == "__main__":
    prog = build_program()
    n_inst = sum(len(blk.instructions) for fn in prog.m.functions
                 for blk in fn.blocks)
    print("built OK; instructions:", n_inst)


# revision 16
# speedup vs baseline: 1.0082x; 1.0082x over previous
"""Cross-attention Trainium2 kernel (8 NeuronCores, SPMD), v6.

Reference computation (per full batch):
  q = x @ Wq + bq;  k = enc @ Wk + bk;  v = enc @ Wv + bv
  att = softmax((q k^T) / sqrt(D));  y = (att v) @ Wo + bo

Sharding: B(=4) x head-group(=2) -> 8 cores. Each core handles one batch
element, 8 of the 16 heads, and ALL 2048 query tokens; it returns the
f32 partial out-projection for its 512 channels. Host sums the two
head-group partials per batch + bo. vs v5's B x T-half split this
halves the duplicated K/V projections and removes the K=128
zero-padding of scores entirely.

Key mechanisms (validated by microbench on this HW):
  * Scores run as row-tiled K=64 matmul PAIRS: even head's k/q on
    partitions 0-63 (PE tile (0,0)), odd head's on 64-127 (tile
    (64,0)), outputs to different PSUM banks -> the two matmuls execute
    CONCURRENTLY (measured 155ns/MM vs 332 serial). The natural channel
    layout already interleaves head pairs this way - no data movement.
  * PE tile-config switches (64-row vs 128-row) cost ~65ns when
    alternating, ~0 when batched: each (pair,tn) unit runs all 8 score
    pairs as one 64-cfg block, then attv + projection thunks as one
    128-cfg block.
  * All matmul operands bf16; N=512 moving (PSUM f32 bank cap,
    walrus rejects 1024). Projections as 8-deep accumulation chains
    (242-261ns/MM measured), out-proj 4-deep.
  * v is built in the att@v lhsT layout vS[u] [128, 2, 8*65] (65-col
    groups per head: [v_h | ones]); psum row 64 of ya = softmax
    denominator for free.
  * Softmax: exp on ACT (psum [128,1024] -> bf16, fused 1/sqrt(D)
    scale), one instruction per (pair,sc) covering both heads;
    reciprocal_approx_fast + gpsimd partition_broadcast + DVE mult.
  * Schedule: fine-grained software pipeline over 16 units
    (tn(4) x head-pair(4)). Each unit weaves 4 "visits" of 2 score
    pairs (64-cfg) with chunks of a 128-cfg micro-op queue holding the
    PREVIOUS unit's att@v chains + projection groups (K/V/Q/O spread
    across units per a hand-placed table in unit_queues()). This keeps
    PE fed while ACT paces the 2-deep scores PSUM, and keeps ACT fed
    across unit boundaries -- v6's block schedule ping-ponged PE<->ACT
    at ~5us/unit. Prologue = 20 HAM warm-up matmuls + K(cp0)+V(0,1)+
    Q(cp0,tn0) under the DMA lead-in; epilogue = attv(last) + out-proj
    of tn3.
  * PSUM: scores 2x[128,1024] (4 banks) + ya [65,1024] (2) +
    projection/out psum 2x[128,512] (2) = exactly 8 banks.

Engine budget per core: PE ~205us busy (the bottleneck: 384 proj slots
+ 128 score pair-slots + 256 attv slots at ~240-310ns measured), ACT
~135us (128 exps of [128,1024] at ~1.05us), DVE ~60us, out-DMA 8MB f32
(host reduces). Measured paired-median ~255us (v5 baseline: 313.7us).
"""

import sys

sys.path.insert(0, "/opt/trn_rl_repo")

import numpy as np

import concourse.bass as bass  # noqa: E402,F401
import concourse.tile as tile  # noqa: E402
from concourse import bacc, mybir  # noqa: E402

F32 = mybir.dt.float32
BF16 = mybir.dt.bfloat16
AF = mybir.ActivationFunctionType

P = 128          # partitions
TC = 2048        # query tokens per core (all of T)
T2 = 1024        # kv sequence length
C = 1024         # embed dim
HC = 8           # heads per core
D = 64           # head dim
NCH = C // P     # 8 input-channel chunks
CP = 4           # output channel-pair chunks (512 cols / 128)
NS = T2 // P     # 8 kv-position chunks
TN = 512         # matmul moving-dim tile
NTN = TC // TN   # 4 query-token chunks
G = D + 1        # v-group stride in vS (64 v cols + ones col)
SCALE = 1.0 / np.sqrt(D)

N_CORES = 8
B_FULL, T_FULL = 4, 2048


def build_program(loop_iters=None, debug=False):
    """loop_iters: if set, wrap the body in a For_i hardware loop (timing)."""
    nc = bacc.Bacc("TRN2", target_bir_lowering=False, debug=False,
                   num_devices=N_CORES)

    aps = {}
    aps["xT"] = nc.dram_tensor("xT", [P, NTN, NCH, TN], BF16,
                               kind="ExternalInput").ap()
    aps["encT"] = nc.dram_tensor("encT", [P, NCH, T2], BF16,
                                 kind="ExternalInput").ap()
    for name in ("wq", "wk", "wv"):
        aps[name] = nc.dram_tensor(name, [P, NCH, CP * P], BF16,
                                   kind="ExternalInput").ap()
    aps["wo"] = nc.dram_tensor("wo", [P, CP, C], BF16,
                               kind="ExternalInput").ap()
    for name in ("bqc", "bkc"):
        aps[name] = nc.dram_tensor(name, [P, CP], F32,
                                   kind="ExternalInput").ap()
    aps["bv"] = nc.dram_tensor("bv", [CP * P], F32,
                               kind="ExternalInput").ap()
    out = nc.dram_tensor("out", [TC, C], F32, kind="ExternalOutput").ap()

    dbg = None
    if debug:
        dbg = {}
        for name, shape, dt in (
                ("d_kS", [P, CP, T2], BF16), ("d_qT", [P, CP, TC], BF16),
                ("d_yT", [P, CP, TC], BF16),
                ("d_vS0", [P, 2, HC * G], BF16),
                ("d_px0", [P, 2 * TN], BF16)):
            dbg[name] = nc.dram_tensor(name, shape, dt,
                                       kind="ExternalOutput").ap()

    with tile.TileContext(nc) as tc:
        if loop_iters is not None:
            with tc.For_i(0, loop_iters, 1):
                _emit(nc, tc, aps, out)
        else:
            _emit(nc, tc, aps, out, dbg)

    nc.compile()
    return nc


def _row(ap):
    return ap.rearrange("(a c) -> a c", a=1)


def _emit(nc, tc, aps, out, dbg=None):
    from contextlib import ExitStack

    with ExitStack() as S:
        pIn = S.enter_context(tc.tile_pool(name="pIn", bufs=1))

        # ---- persistent tiles
        wo = pIn.tile([P, CP, C], BF16, tag="wo", name="wo")
        kS = pIn.tile([P, CP, T2], BF16, tag="kS", name="kS")
        qT = pIn.tile([P, CP, TC], BF16, tag="qT", name="qT")
        yT = pIn.tile([P, CP, TC], BF16, tag="yT", name="yT")
        vS = [pIn.tile([P, 2, HC * G], BF16, tag=f"vS_{u}", name=f"vS_{u}")
              for u in range(NS // 2)]
        xTs = pIn.tile([P, NTN, NCH, TN], BF16, tag="xTs", name="xTs")
        wq = pIn.tile([P, NCH, CP * P], BF16, tag="wq", name="wq")
        bqc = pIn.tile([P, CP], F32, tag="bqc", name="bqc")

        psP = S.enter_context(tc.tile_pool(name="psP", bufs=2, space="PSUM"))
        psS = S.enter_context(tc.tile_pool(name="psS", bufs=2, space="PSUM"))
        psY = S.enter_context(tc.tile_pool(name="psY", bufs=1, space="PSUM"))

        state = {}

        def open_proj_scope(S2):
            # DMA need-order: K-proj inputs first (encT+wk), then wv, then
            # xT tn0 + wq (Q-proj), then rest of xT, wo last.
            pTmp = S2.enter_context(tc.tile_pool(name="pTmp", bufs=1))
            bkc = pTmp.tile([P, CP], F32, tag="bkc", name="bkc")
            nc.sync.dma_start(out=bkc, in_=aps["bkc"])
            nc.scalar.dma_start(out=bqc, in_=aps["bqc"])
            bv_row = pTmp.tile([1, CP * P], F32, tag="bv_row", name="bv_row")
            nc.sync.dma_start(out=bv_row, in_=_row(aps["bv"]))

            encT = pTmp.tile([P, NCH, T2], BF16, tag="encT", name="encT")
            wk = pTmp.tile([P, NCH, CP * P], BF16, tag="wk", name="wk")
            wv = pTmp.tile([P, NCH, CP * P], BF16, tag="wv", name="wv")
            for kc in range(NCH):
                # encT split across both queues, wk woven in kc order so
                # the K-proj chain starts as soon as chunk 0 lands
                qe = nc.sync if kc % 2 == 0 else nc.scalar
                qw = nc.scalar if kc % 2 == 0 else nc.sync
                qe.dma_start(out=encT[:, kc, :], in_=aps["encT"][:, kc, :])
                qw.dma_start(out=wk[:, kc, :], in_=aps["wk"][:, kc, :])
            for kc in range(NCH):
                q = nc.sync if kc >= 4 else nc.scalar
                q.dma_start(out=wv[:, kc, :], in_=aps["wv"][:, kc, :])
            for kc in range(NCH):
                nc.sync.dma_start(out=xTs[:, 0, kc, :],
                                  in_=aps["xT"][:, 0, kc, :])
                nc.scalar.dma_start(out=wq[:, kc, :], in_=aps["wq"][:, kc, :])
            for tn in range(1, NTN):
                for kc in range(NCH):
                    q = nc.sync if (kc + tn) % 2 else nc.scalar
                    q.dma_start(out=xTs[:, tn, kc, :],
                                in_=aps["xT"][:, tn, kc, :])
            nc.scalar.dma_start(out=wo, in_=aps["wo"])

            bvb = pTmp.tile([P, CP * P], F32, tag="bvb", name="bvb")
            nc.gpsimd.partition_broadcast(bvb, bv_row)
            state.update(encT=encT, wk=wk, wv=wv, bkc=bkc, bvb=bvb)

        def chain_ops(n_mm, mk_ps, mm, tail):
            """Micro-op list for an accumulation-chain group: n_mm matmul
            callables sharing one psum tile (allocated at first call) plus a
            trailing non-PE consumer."""
            box = {}

            def mm_i(i):
                if i == 0:
                    box["ps"] = mk_ps()
                mm(box["ps"], i)
            return ([lambda i=i: mm_i(i) for i in range(n_mm)]
                    + [lambda: tail(box["ps"])])

        def k_ops(cp, stn):
            def mm(ps, kc):
                nc.tensor.matmul(
                    ps, state["wk"][:, kc, cp * P:(cp + 1) * P],
                    state["encT"][:, kc, stn * TN:(stn + 1) * TN],
                    start=(kc == 0), stop=(kc == NCH - 1))

            def tail(ps):
                nc.vector.tensor_scalar_add(
                    kS[:, cp, stn * TN:(stn + 1) * TN], ps,
                    state["bkc"][:, cp:cp + 1])
            return chain_ops(
                NCH, lambda: psP.tile([P, TN], F32, tag="pp", name="psK"),
                mm, tail)

        def q_ops(cp, tn):
            def mm(ps, kc):
                nc.tensor.matmul(
                    ps, wq[:, kc, cp * P:(cp + 1) * P], xTs[:, tn, kc, :],
                    start=(kc == 0), stop=(kc == NCH - 1))

            def tail(ps):
                nc.vector.tensor_scalar_add(
                    qT[:, cp, tn * TN:(tn + 1) * TN], ps, bqc[:, cp:cp + 1])
            return chain_ops(
                NCH, lambda: psP.tile([P, TN], F32, tag="pp", name="psQ"),
                mm, tail)

        def v_ops(sc):
            u, j = sc // 2, sc % 2

            def mm(ps, kc):
                nc.tensor.matmul(
                    ps, state["encT"][:, kc, sc * P:(sc + 1) * P],
                    state["wv"][:, kc, :],
                    start=(kc == 0), stop=(kc == NCH - 1))

            def tail(ps):
                dst = vS[u][:, j, :].rearrange(
                    "p (h g) -> p h g", g=G)[:, :, 0:D]
                srcp = ps.rearrange("p (h g) -> p h g", g=D)
                bsrc = state["bvb"].rearrange("p (h g) -> p h g", g=D)
                nc.vector.tensor_add(dst, srcp, bsrc)
            return chain_ops(
                NCH, lambda: psP.tile([P, TN], F32, tag="pp", name="psV"),
                mm, tail)

        def o_ops(tp, nn):
            # f32 partial out-projection (host adds the other head-group
            # partial + bo); nn==1 tail also DMAs the finished row block.
            def mm(ps, kc):
                nc.tensor.matmul(
                    ps, yT[:, kc, tp * P:(tp + 1) * P],
                    wo[:, kc, nn * TN:(nn + 1) * TN],
                    start=(kc == 0), stop=(kc == CP - 1))

            def tail(ps):
                if nn == 0:
                    state[f"osb{tp}"] = state["pO"].tile(
                        [P, C], F32, tag="o", bufs=2, name=f"o{tp}")
                o_sb = state[f"osb{tp}"]
                nc.vector.tensor_copy(o_sb[:, nn * TN:(nn + 1) * TN], ps)
                if nn == 1:
                    nc.scalar.dma_start(
                        out=out[tp * P:(tp + 1) * P, :], in_=o_sb)
            return chain_ops(
                CP, lambda: psP.tile([P, TN], F32, tag="pp", name="psO"),
                mm, tail)

        def attv_ops(u):
            """att@v chains + normalization for unit u (consumes px[u])."""
            tn, cp = u // CP, u % CP
            ts = slice(tn * TN, (tn + 1) * TN)
            h0, h1 = 2 * cp, 2 * cp + 1
            box = {}
            ops = []

            def av(sc, half, h):
                if sc == 0 and half == 0:
                    box["ya"] = psY.tile([G, 2 * TN], F32, tag="ya",
                                         name="ya")
                uu, j = sc // 2, sc % 2
                nc.tensor.matmul(
                    box["ya"][:, half * TN:(half + 1) * TN],
                    vS[uu][:, j, h * G:(h + 1) * G],
                    state["px"][u][sc][:, half * TN:(half + 1) * TN],
                    start=(sc == 0), stop=(sc == NS - 1))
            for sc in range(NS):
                ops.append(lambda sc=sc: av(sc, 0, h0))
                ops.append(lambda sc=sc: av(sc, 1, h1))

            def norm():
                ya = box["ya"]
                den = pRc.tile([1, 2 * TN], F32, tag="den", bufs=2,
                               name="den")
                nc.vector.tensor_copy(den, ya[D:D + 1, :])
                rc = pRc.tile([1, 2 * TN], F32, tag="rc", bufs=2, name="rc")
                nc.vector.reciprocal_approx_fast(rc, den)
                bc = pBc.tile([D, 2 * TN], F32, tag="bc", bufs=2, name="bc")
                nc.gpsimd.partition_broadcast(bc, rc)
                nc.vector.tensor_mul(yT[0:D, cp, ts], ya[0:D, 0:TN],
                                     bc[:, 0:TN])
                nc.vector.tensor_mul(yT[D:P, cp, ts], ya[0:D, TN:2 * TN],
                                     bc[:, TN:2 * TN])
                del state["px"][u]
            ops.append(norm)
            return ops

        def emit_unit(u, queue):
            """Scores for unit u woven with 128-cfg micro-ops from queue.

            Visits issue 2 score-pairs (4 MMs, 64-cfg) + their exps, then
            drain a slice of the queue (attv of unit u-1 + projection
            groups), keeping PE fed while ACT paces the scores banks."""
            tn, cp = u // CP, u % CP
            ts = slice(tn * TN, (tn + 1) * TN)
            px_list = []
            state["px"][u] = px_list
            # 2 score-pairs per visit measured best (1/visit: +7us switch
            # cost; 4/visit: +7us PSUM-bank gating stalls)
            n_visits = NS // 2
            for v in range(n_visits):
                for sc in (2 * v, 2 * v + 1):
                    ps = psS.tile([P, 2 * TN], F32, tag="ps", bufs=2,
                                  name="psS")
                    nc.tensor.matmul(
                        ps[:, 0:TN], kS[0:D, cp, sc * P:(sc + 1) * P],
                        qT[0:D, cp, ts], start=True, stop=True)
                    nc.tensor.matmul(
                        ps[:, TN:2 * TN], kS[D:P, cp, sc * P:(sc + 1) * P],
                        qT[D:P, cp, ts], start=True, stop=True)
                    px = pPx.tile([P, 2 * TN], BF16, tag="px", bufs=18,
                                  name="px")
                    nc.scalar.activation(px, ps, AF.Exp, scale=float(SCALE))
                    px_list.append(px)
                n_chunk = (len(queue) + n_visits - v - 1) // (n_visits - v)
                for _ in range(n_chunk):
                    queue.pop(0)()
            while queue:
                queue.pop(0)()

        # ================= schedule =================
        with ExitStack() as S2:
            open_proj_scope(S2)
            pPx = S2.enter_context(tc.tile_pool(name="pPx", bufs=18))
            pRc = S2.enter_context(tc.tile_pool(name="pRc", bufs=2))
            pBc = S2.enter_context(tc.tile_pool(name="pBc", bufs=2))
            state["pO"] = S2.enter_context(tc.tile_pool(name="pO", bufs=2))
            state["px"] = {}

            for u in range(NS // 2):
                ones_dst = vS[u].rearrange(
                    "p two (h g) -> p (two h) g", g=G)[:, :, D:D + 1]
                nc.vector.memset(ones_dst, 1.0)

            # HAM warm-up: dummy matmuls on uninitialized SBUF while the
            # first DMAs land, so the PE clock gate is at 2.4GHz (not the
            # cold 1.2GHz) when real work starts. Results are discarded.
            for i in range(20):
                ps = psS.tile([P, 2 * TN], F32, tag="ps", bufs=2,
                              name="psWarm")
                nc.tensor.matmul(ps[:, 0:TN], qT[:, 0, 0:P],
                                 qT[:, 0, 0:TN], start=True, stop=True)

            # prologue: the minimal serial work before unit 0's scores,
            # ordered by DMA arrival (encT+wk first, wv, xT tn0 last)
            for op in (k_ops(0, 0) + k_ops(0, 1) + v_ops(0) + v_ops(1)
                       + q_ops(0, 0)):
                op()

            # per-unit woven thunk groups (pre = before attv(u-1) in the
            # queue -- needed for V groups feeding that attv; post = after).
            Qs, Ks, Vs, Os = q_ops, k_ops, v_ops, o_ops

            def unit_queues():
                q = {}
                q[0] = (Vs(2) + Vs(3) + Ks(1, 0) + Ks(1, 1) + Qs(1, 0), [])
                q[1] = (Vs(4) + Vs(5) + Vs(6) + Vs(7),
                        Ks(2, 0) + Ks(2, 1) + Qs(2, 0))
                q[2] = ([], Ks(3, 0) + Ks(3, 1) + Qs(3, 0))
                q[3] = ([], Qs(0, 1))
                q[4] = ([], Qs(1, 1) + Qs(2, 1))
                q[5] = ([], Qs(3, 1) + Os(0, 0) + Os(0, 1))
                q[6] = ([], Os(1, 0) + Os(1, 1) + Qs(0, 2))
                q[7] = ([], Os(2, 0) + Os(2, 1))
                q[8] = ([], Os(3, 0) + Os(3, 1) + Qs(1, 2))
                q[9] = ([], Qs(2, 2) + Qs(3, 2) + Os(4, 0))
                q[10] = ([], Os(4, 1) + Os(5, 0) + Os(5, 1))
                q[11] = ([], Os(6, 0) + Os(6, 1) + Qs(0, 3))
                q[12] = ([], Os(7, 0) + Os(7, 1) + Qs(1, 3))
                q[13] = ([], Qs(2, 3) + Qs(3, 3) + Os(8, 0))
                q[14] = ([], Os(8, 1) + Os(9, 0) + Os(9, 1))
                q[15] = ([], Os(10, 0) + Os(10, 1) + Os(11, 0) + Os(11, 1))
                return q

            queues = unit_queues()
            for u in range(4 * CP):
                pre, post = queues[u]
                av = attv_ops(u - 1) if u >= 1 else []
                emit_unit(u, pre + av + post)

            if dbg is not None:
                nc.sync.dma_start(out=dbg["d_px0"], in_=state["px"][15][0])
                nc.sync.dma_start(out=dbg["d_vS0"], in_=vS[0])
                nc.sync.dma_start(out=dbg["d_kS"], in_=kS)
                nc.sync.dma_start(out=dbg["d_qT"], in_=qT)

            # epilogue: last unit's attv + the final token chunk's out-proj
            for op in attv_ops(4 * CP - 1):
                op()
            for tp in range(12, 16):
                for op in Os(tp, 0) + Os(tp, 1):
                    op()

            if dbg is not None:
                nc.sync.dma_start(out=dbg["d_yT"], in_=yT)


def make_in_maps(inputs):
    """Full fp32 inputs -> per-core input maps (host-side shard + layout)."""
    import ml_dtypes

    bf16 = ml_dtypes.bfloat16
    x = np.asarray(inputs["x"], dtype=np.float32)
    enc = np.asarray(inputs["enc_x"], dtype=np.float32)

    def chunked_T(a2d):
        # [rows, C] fp32 -> [128, NCH, rows] bf16: [p, c, r] = a[r, c*128+p]
        t = np.ascontiguousarray(
            a2d.T.reshape(NCH, P, a2d.shape[0]).transpose(1, 0, 2))
        return t.astype(bf16)

    encT = [chunked_T(enc[b]) for b in range(x.shape[0])]
    # xT[p, tn, kc, j] = x[b, tn*512+j, kc*128+p]
    xTb = [np.ascontiguousarray(
        x[b].reshape(NTN, TN, NCH, P).transpose(3, 0, 2, 1)).astype(bf16)
        for b in range(x.shape[0])]

    Wq = np.asarray(inputs["Wq"], np.float32)
    Wk = np.asarray(inputs["Wk"], np.float32)
    Wv = np.asarray(inputs["Wv"], np.float32)
    Wo = np.asarray(inputs["Wo"], np.float32)
    bq = np.asarray(inputs["bq"], np.float32)
    bk = np.asarray(inputs["bk"], np.float32)
    bv = np.asarray(inputs["bv"], np.float32)

    hgw = []
    for hg in range(2):
        cs = slice(hg * CP * P, (hg + 1) * CP * P)
        m = {}
        for name, W in (("wq", Wq), ("wk", Wk), ("wv", Wv)):
            m[name] = np.ascontiguousarray(
                W[:, cs].reshape(NCH, P, CP * P).transpose(1, 0, 2)
            ).astype(bf16)
        m["wo"] = np.ascontiguousarray(
            Wo[cs, :].reshape(CP, P, C).transpose(1, 0, 2)).astype(bf16)
        m["bqc"] = np.ascontiguousarray(bq[cs].reshape(CP, P).T)
        m["bkc"] = np.ascontiguousarray(bk[cs].reshape(CP, P).T)
        m["bv"] = np.ascontiguousarray(bv[cs])
        hgw.append(m)

    maps = []
    for core in range(N_CORES):
        b, hg = core // 2, core % 2
        m = {"xT": xTb[b], "encT": encT[b]}
        m.update(hgw[hg])
        maps.append(m)
    return maps


_CACHED = None


def _get_program():
    global _CACHED
    if _CACHED is None:
        _CACHED = build_program()
    return _CACHED


def kernel(**inputs):
    x = np.asarray(inputs["x"], dtype=np.float32)
    B, T, Cx = x.shape
    assert (B, T, Cx) == (B_FULL, T_FULL, C), (B, T, Cx)

    nc = _get_program()
    in_maps = make_in_maps(inputs)

    from concourse.bass_utils import run_bass_kernel_spmd
    res = None
    last_err = None
    for _attempt in range(3):
        try:
            res = run_bass_kernel_spmd(nc, in_maps,
                                       core_ids=list(range(N_CORES)))
            break
        except Exception as e:  # transient NRT/axon failures: retry
            last_err = e
    if res is None:
        raise last_err

    bo = np.asarray(inputs["bo"], np.float32)
    outp = np.empty((B, T, C), dtype=np.float32)
    for b in range(B):
        outp[b] = (res.results[2 * b]["out"] + res.results[2 * b + 1]["out"]
                   + bo)
    return outp


if __name__ == "__main__":
    prog = build_program()
    n_inst = sum(len(blk.instructions) for fn in prog.m.functions
                 for blk in fn.blocks)
    print("built OK; instructions:", n_inst)
